# revision 1
# baseline (speedup 1.0000x reference)
"""BiGRU encoder on 8 Trainium2 NeuronCores.

Strategy: the T=2048 recurrence is split into 32 chunks per direction of 64
steps each, computed in parallel as independent chains with a W-step warm-up
prefix (the GRU state's dependence on its past decays geometrically; W=48
gives ~1e-4 relative error vs an exact scan). Cores 0-3 run the forward
direction (8 chains x 16 batch = 128 rows each), cores 4-7 the backward
direction on host-reversed data. Per step, each core does:
  gates = [x_t | h_{t-1}] @ [Wih | Whh]^T  as f32r matmuls (stationary = xT /
  hT chunks of 128 rows, moving = weight tiles [128,512]), accumulated in
  PSUM; sigmoid/tanh on ACT; elementwise GRU update on DVE; h is transposed
  for the next step's matmul with PE-transpose.
The host slices x, builds the per-core layouts, and reassembles the output.
"""
import os
import sys
import numpy as np

try:
    import concourse.bass as bass
except ImportError:
    import sys
    sys.path.insert(0, "/opt/trn_rl_repo")
    import concourse.bass as bass

import concourse.tile as tile
from concourse import bacc, mybir
from concourse.bass_utils import run_bass_kernel_spmd

F32 = mybir.dt.float32
F32R = mybir.dt.float32r

# geometry (hardcoded for this problem)
B = 16          # batch
T = 2048        # timesteps
F = 512         # hidden/feature size
KC = 4          # contraction chunks (F / 128)
CHUNK = int(os.environ.get("GRU_CHUNK", "64"))   # stored steps per chain
WARM = int(os.environ.get("GRU_WARM", "48"))     # warm-up steps per chain
S = CHUNK + WARM                                  # total steps per core
NCH = 8         # chains per core
R = NCH * B     # rows per core = 128
N_CORES = 8
N_FWD = 4       # cores 0..3 forward, 4..7 backward
ACT = mybir.ActivationFunctionType
ALU = mybir.AluOpType

_PROG_CACHE = {}


def _round_f32r(a: np.ndarray) -> np.ndarray:
    """Round fp32 to the f32r grid (round-to-nearest, 12 explicit mantissa
    bits) so data fed to f32r matmuls matches what the PE consumes."""
    u = np.ascontiguousarray(a, np.float32).view(np.uint32).astype(np.uint64)
    u = (u + (1 << 10)) & np.uint64(0xFFFFF800)
    return u.astype(np.uint32).view(np.float32)


def _build_program(has_bias: bool):
    nc = bacc.Bacc("TRN2", target_bir_lowering=False, debug=False)

    xT_d = nc.dram_tensor("xT", [S, 128, KC, 128], F32R, kind="ExternalInput").ap()
    xr_d = nc.dram_tensor("xr", [S, 128, F], F32, kind="ExternalInput").ap()
    wih_d = nc.dram_tensor("wih", [128, KC, 3 * F], F32R, kind="ExternalInput").ap()
    whh_d = nc.dram_tensor("whh", [128, KC, 3 * F], F32R, kind="ExternalInput").ap()
    ident_d = nc.dram_tensor("ident", [128, 128], F32, kind="ExternalInput").ap()
    if has_bias:
        # row vectors: [1, 3F] each; bias_i enters gi (r,z,n), bias_h enters
        # gh (r,z,n). r/z parts can be summed; the n parts must stay separate.
        bias_i_d = nc.dram_tensor("bias_i", [1, 3 * F], F32R, kind="ExternalInput").ap()
        bias_h_d = nc.dram_tensor("bias_h", [1, 3 * F], F32R, kind="ExternalInput").ap()
        ones_d = nc.dram_tensor("ones", [1, 128], F32R, kind="ExternalInput").ap()
    out_d = nc.dram_tensor("out", [CHUNK, 128, F], F32, kind="ExternalOutput").ap()

    with tile.TileContext(nc) as tc:
        with (
            tc.tile_pool(name="const", bufs=1) as constp,
            tc.tile_pool(name="xs", bufs=1) as xsp,
            tc.tile_pool(name="ew", bufs=1) as ewp,
            tc.tile_pool(name="ps", bufs=1, space="PSUM") as psp,
        ):
            wih = constp.tile([128, KC, 3 * F], F32R, name="wih_sb")
            nc.sync.dma_start(wih[:], wih_d[:])
            whh = constp.tile([128, KC, 3 * F], F32R, name="whh_sb")
            nc.sync.dma_start(whh[:], whh_d[:])
            ident = constp.tile([128, 128], F32, name="ident_sb")
            nc.sync.dma_start(ident[:], ident_d[:])
            if has_bias:
                bias_i = constp.tile([1, 3 * F], F32R, name="bias_i_sb")
                nc.sync.dma_start(bias_i[:], bias_i_d[:])
                bias_h = constp.tile([1, 3 * F], F32R, name="bias_h_sb")
                nc.sync.dma_start(bias_h[:], bias_h_d[:])
                ones = constp.tile([1, 128], F32R, name="ones_sb")
                nc.sync.dma_start(ones[:], ones_d[:])

            def load_xT(s):
                xT_t = xsp.tile([128, KC, 128], F32R, name="xT_t", tag="xT_t", bufs=5)
                nc.sync.dma_start(xT_t[:], xT_d[s])
                return xT_t

            def load_xr(s):
                xr_t = xsp.tile([128, F], F32, name="xr_t", tag="xr_t", bufs=4)
                nc.sync.dma_start(xr_t[:], xr_d[s])
                return xr_t

            def gi_r_mms(s, xT_t, final):
                """r-gate part of x_t @ Wih^T — emitted two steps ahead so the
                PE has fill work while the elementwise chain finishes."""
                r_ps = psp.tile([128, F], F32, name="r_ps", tag="r_ps", bufs=3)
                for kc in range(KC):
                    nc.tensor.matmul(
                        r_ps[:], xT_t[:, kc, :], wih[:, kc, 0:F],
                        start=(kc == 0),
                        stop=final and (kc == KC - 1) and not has_bias)
                if has_bias:
                    nc.tensor.matmul(r_ps[:], ones[:], bias_i[:, 0:F],
                                     start=False, stop=final)
                return r_ps

            def gi_zinn_mms(s, xT_t, final):
                """z/n parts of x_t @ Wih^T (+ bias); inn shares a psum tag
                with the transpose scratch (their live ranges alternate)."""
                z_ps = psp.tile([128, F], F32, name="z_ps", tag="z_ps", bufs=2)
                inn_ps = psp.tile([128, F], F32, name="inn_ps", tag="inn_tr", bufs=2)
                for j, dst in ((1, z_ps), (2, inn_ps)):
                    lo = j * F
                    for kc in range(KC):
                        nc.tensor.matmul(
                            dst[:], xT_t[:, kc, :], wih[:, kc, lo:lo + F],
                            start=(kc == 0),
                            stop=final and (kc == KC - 1) and not has_bias,
                        )
                    if has_bias:
                        nc.tensor.matmul(
                            dst[:], ones[:], bias_i[:, lo:lo + F],
                            start=False, stop=final,
                        )
                return z_ps, inn_ps

            def recurrent_mms(h2, r_ps, z_ps):
                """Transpose h_{t-1} (PE) and run h @ Whh^T. PE emission is
                interleaved with hT availability: h2 half0 (kc 0/1) unblocks
                its transposes and the first r/hn matmuls while half1 of the
                elementwise chain is still finishing."""
                tr_ps = psp.tile([128, KC, 128], F32, name="tr_ps", tag="inn_tr", bufs=2)
                hT_t = ewp.tile([128, KC, 128], F32R, name="hT_t", tag="hT_t", bufs=2)
                hn_ps = psp.tile([128, F], F32, name="hn_ps", tag="hn_ps", bufs=1)
                H = F // 2

                def tr(kc):
                    # the 4 transposes share one PSUM bank (one zero-region):
                    # start only on the first; disjoint quarters written
                    nc.tensor.matmul(
                        tr_ps[:, kc, :], h2[:, kc * 128:(kc + 1) * 128], ident[:],
                        is_transpose=True, start=(kc == 0), stop=(kc == KC - 1))
                    nc.scalar.copy(hT_t[:, kc, :], tr_ps[:, kc, :])

                def mm(dst, kc, lo, n, start, stop):
                    nc.tensor.matmul(
                        dst, hT_t[:, kc, :], whh[:, kc, lo:lo + n],
                        start=start, stop=stop and not has_bias)

                tr(0); tr(1)
                mm(r_ps[:], 0, 0, F, False, False)
                mm(r_ps[:], 1, 0, F, False, False)
                mm(hn_ps[:, 0:H], 0, 2 * F, H, True, False)
                mm(hn_ps[:, 0:H], 1, 2 * F, H, False, False)
                tr(2); tr(3)
                mm(r_ps[:], 2, 0, F, False, False)
                mm(r_ps[:], 3, 0, F, False, True)
                mm(hn_ps[:, 0:H], 2, 2 * F, H, False, False)
                mm(hn_ps[:, 0:H], 3, 2 * F, H, False, True)
                for kc in range(KC):
                    mm(hn_ps[:, H:F], kc, 2 * F + H, H, False, kc == KC - 1)
                for kc in range(KC):
                    mm(z_ps[:], kc, F, F, False, kc == KC - 1)
                if has_bias:
                    nc.tensor.matmul(r_ps[:], ones[:], bias_h[:, 0:F],
                                     start=False, stop=True)
                    nc.tensor.matmul(z_ps[:], ones[:], bias_h[:, F:2 * F],
                                     start=False, stop=True)
                    for half in range(2):
                        lo = 2 * F + half * H
                        nc.tensor.matmul(
                            hn_ps[:, half * H:(half + 1) * H], ones[:],
                            bias_h[:, lo:lo + H], start=False, stop=True)
                return hn_ps
                return hT_t

            # ---- main loop ----
            xT_tiles = {0: load_xT(0), 1: load_xT(1)}
            xr_t = load_xr(0)
            r_tiles = {0: gi_r_mms(0, xT_tiles[0], final=True)}
            zinn = gi_zinn_mms(0, xT_tiles[0], final=True)
            r_tiles[1] = gi_r_mms(1, xT_tiles[1], final=False)
            h2_prev = None
            for s in range(S):
                r_ps = r_tiles.pop(s)
                z_ps, inn_ps = zinn
                if s > 0:
                    hn_ps = recurrent_mms(h2_prev, r_ps, z_ps)

                H = F // 2
                r_s = ewp.tile([128, F], F32, name="r_s", tag="r_s", bufs=2)
                nc.scalar.activation(r_s[:, 0:H], r_ps[:, 0:H], ACT.Sigmoid)
                nc.scalar.activation(r_s[:, H:F], r_ps[:, H:F], ACT.Sigmoid)
                z_s = ewp.tile([128, F], F32, name="z_s", tag="z_s", bufs=2)
                nc.scalar.activation(z_s[:], z_ps[:], ACT.Sigmoid)

                # independent of n (overlaps the n chain):
                # u = 1-z = sigmoid(-z_pre) ; q = z*h + x
                u_s = ewp.tile([128, F], F32, name="u_s", tag="u_s", bufs=2)
                nc.scalar.activation(u_s[:], z_ps[:], ACT.Sigmoid, scale=-1.0)
                if s > 0:
                    zh = ewp.tile([128, F], F32, name="zh", tag="zh", bufs=2)
                    nc.vector.tensor_mul(zh[:], z_s[:], h2_prev[:])
                    q_s = ewp.tile([128, F], F32, name="q_s", tag="q_s", bufs=2)
                    nc.vector.tensor_add(q_s[:], zh[:], xr_t[:])
                else:
                    q_s = xr_t

                # n chain + h2, halved along features so the next step's
                # transposes/matmuls start on half 0 while half 1 finishes
                h2 = ewp.tile([128, F], F32, name="h2", tag="h2", bufs=3)
                for hh in range(2):
                    sl = slice(hh * H, (hh + 1) * H)
                    if s > 0:
                        rhn = ewp.tile([128, H], F32, name="rhn", tag="rhn", bufs=3)
                        nc.vector.tensor_mul(rhn[:], r_s[:, sl], hn_ps[:, sl])
                        npre = ewp.tile([128, H], F32, name="npre", tag="npre", bufs=3)
                        nc.vector.tensor_add(npre[:], rhn[:], inn_ps[:, sl])
                        n_in = npre[:]
                    else:
                        n_in = inn_ps[:, sl]
                    n_s = ewp.tile([128, H], F32, name="n_s", tag="n_s", bufs=3)
                    nc.scalar.activation(n_s[:], n_in, ACT.Tanh)
                    un = ewp.tile([128, H], F32, name="un", tag="un", bufs=3)
                    nc.vector.tensor_mul(un[:], u_s[:, sl], n_s[:])
                    # h2 written in quarters: each unblocks its transpose
                    for qq in range(2):
                        qsl = slice(hh * H + qq * 128, hh * H + (qq + 1) * 128)
                        usl = slice(qq * 128, (qq + 1) * 128)
                        nc.vector.tensor_add(h2[:, qsl], un[:, usl], q_s[:, qsl])

                # prefetch + next-step gi fill the PE while the
                # elementwise chain runs; r two steps ahead
                if s + 1 < S:
                    xr_t2 = load_xr(s + 1)
                    zinn = gi_zinn_mms(s + 1, xT_tiles[s + 1], final=False)
                if s + 2 < S:
                    xT_tiles[s + 2] = load_xT(s + 2)
                    r_tiles[s + 2] = gi_r_mms(s + 2, xT_tiles[s + 2], final=False)
                xT_tiles.pop(s, None)

                if s >= WARM:
                    nc.sync.dma_start(out_d[s - WARM], h2[:])
                h2_prev = h2
                if s + 1 < S:
                    xr_t = xr_t2

    nc.compile()
    return nc


def _prep_core_inputs(cx, Wih, Whh, bih, bhh, core):
    """Build the per-core input map. cx: [B, T, F] fp32."""
    fwd = core < N_FWD
    k = core if fwd else core - N_FWD
    c = np.arange(NCH)
    g = NCH * k + c                                   # global chunk ids
    s = np.arange(S)
    if fwd:
        t_idx = (CHUNK * g[:, None] - WARM) + s[None, :]       # [NCH, S]
    else:
        tau = (CHUNK * g[:, None] - WARM) + s[None, :]
        t_idx = (T - 1) - tau
    valid = (t_idx >= 0) & (t_idx < T)
    t_safe = np.clip(t_idx, 0, T - 1)
    # xc[b, c, s, f]
    xc = cx[:, t_safe, :]                              # [B, NCH, S, F]
    xc = xc * valid[None, :, :, None]
    xr = np.ascontiguousarray(
        xc.transpose(2, 1, 0, 3).reshape(S, R, F), np.float32)  # [S, c*16+b, F]
    xT = np.ascontiguousarray(
        xr.reshape(S, R, KC, 128).transpose(0, 3, 2, 1))        # [S, p2, kc, r]
    Wt = np.ascontiguousarray(Wih.T.reshape(KC, 128, 3 * F).transpose(1, 0, 2))
    Ht = np.ascontiguousarray(Whh.T.reshape(KC, 128, 3 * F).transpose(1, 0, 2))
    m = {
        "xT": _round_f32r(xT),
        "xr": xr,
        "wih": _round_f32r(Wt),
        "whh": _round_f32r(Ht),
        "ident": np.eye(128, dtype=np.float32),
    }
    if bih is not None:
        m["bias_i"] = _round_f32r(bih.reshape(1, 3 * F))
        m["bias_h"] = _round_f32r(bhh.reshape(1, 3 * F))
        m["ones"] = _round_f32r(np.ones((1, 128), np.float32))
    return m


def _install_ntff_hook():
    """The agent image's antenv lacks axon_hooks; recreate it so
    run_bass_kernel_spmd(trace=True) can capture NTFF profiles."""
    import sys as _sys
    if "antenv.axon_hooks" in _sys.modules:
        return True
    so_path = "/opt/axon/libaxon_pjrt.so"
    if not os.path.exists(so_path):
        return False
    import contextlib
    import ctypes
    import types
    lib = ctypes.CDLL(so_path)
    if not hasattr(lib, "axon_start_nrt_profile"):
        return False
    lib.axon_start_nrt_profile.argtypes = [
        ctypes.POINTER(ctypes.c_int64), ctypes.c_size_t]
    lib.axon_start_nrt_profile.restype = ctypes.c_int64
    lib.axon_stop_nrt_profile.argtypes = [ctypes.c_char_p]
    lib.axon_stop_nrt_profile.restype = ctypes.c_int64

    @contextlib.contextmanager
    def _hook(output_dir, device_ids):
        import jax
        jax.devices()
        if device_ids:
            ids = (ctypes.c_int64 * len(device_ids))(*device_ids)
            rc = lib.axon_start_nrt_profile(ids, len(device_ids))
        else:
            rc = lib.axon_start_nrt_profile(None, 0)
        if rc != 0:
            raise RuntimeError(f"axon_start_nrt_profile rc={rc}")
        try:
            yield
        finally:
            n = lib.axon_stop_nrt_profile(str(output_dir).encode())
            print(f"profile: {n} file(s) written to {output_dir}",
                  file=sys.stderr)

    mod = types.ModuleType("antenv.axon_hooks")
    mod.get_axon_ntff_profile_hook = lambda: _hook
    mod.set_axon_ntff_profile_hook = lambda h: None
    _sys.modules["antenv.axon_hooks"] = mod
    return True


def _run(inputs, trace=False):
    input_x = np.asarray(inputs["input_x"], np.float32)
    Wih_f = np.asarray(inputs["Wih_f"], np.float32)
    Whh_f = np.asarray(inputs["Whh_f"], np.float32)
    Wih_b = np.asarray(inputs["Wih_b"], np.float32)
    Whh_b = np.asarray(inputs["Whh_b"], np.float32)
    bih_f = np.asarray(inputs["bih_f"], np.float32)
    bhh_f = np.asarray(inputs["bhh_f"], np.float32)
    bih_b = np.asarray(inputs["bih_b"], np.float32)
    bhh_b = np.asarray(inputs["bhh_b"], np.float32)
    L = int(inputs["L"])

    has_bias = bool(
        np.any(bih_f) or np.any(bhh_f) or np.any(bih_b) or np.any(bhh_b))
    key = (has_bias, S, CHUNK)
    if key not in _PROG_CACHE:
        _PROG_CACHE[key] = _build_program(has_bias)
    nc = _PROG_CACHE[key]

    cx = np.ascontiguousarray(input_x[:, :, :F])
    in_maps = []
    for core in range(N_CORES):
        fwd = core < N_FWD
        in_maps.append(_prep_core_inputs(
            cx,
            Wih_f if fwd else Wih_b,
            Whh_f if fwd else Whh_b,
            (bih_f if fwd else bih_b) if has_bias else None,
            (bhh_f if fwd else bhh_b) if has_bias else None,
            core,
        ))

    if trace and not _install_ntff_hook():
        trace = False
    res = run_bass_kernel_spmd(nc, in_maps, list(range(N_CORES)), trace=trace)

    # reassemble: hs[dir][b, t, F]
    hs_f = np.empty((B, T, F), np.float32)
    hs_b = np.empty((B, T, F), np.float32)
    for core in range(N_CORES):
        o = res.results[core]["out"].reshape(CHUNK, NCH, B, F)
        o = o.transpose(1, 2, 0, 3)                    # [c, b, chunk, F]
        fwd = core < N_FWD
        k = core if fwd else core - N_FWD
        dst = hs_f if fwd else hs_b
        for c in range(NCH):
            t0 = CHUNK * (NCH * k + c)
            dst[:, t0:t0 + CHUNK, :] = o[c]
    out = np.empty((B, T - 2 * L, 2 * F), np.float32)
    out[:, :, :F] = hs_f[:, L:T - L, :]
    out[:, :, F:] = hs_b[:, L:T - L, :]
    return out, res


def kernel(**inputs) -> np.ndarray:
    out, _ = _run(inputs, trace=False)
    return out

